# revision 42
# baseline (speedup 1.0000x reference)
"""Multi-head attention (nn_MultiHeadAttention) on 8 Trainium2 NeuronCores.

Hybrid batch x head sharding: core c owns batch c//4 and heads
4*(c%4)..4*(c%4)+3 (two head-PAIRS). Each core computes its 4 heads' full
attention plus the partial output projection for its batch; the host sums
the 4 partials per batch and adds bo.

Per-core kernel phases:
  P1  q/k projections for both pairs -> qhT/khT [128(dkA|dkB), S] bf16;
      qt streams on the Activation DGE queue, kt on Sync (parallel issue).
  P2  v projection as vhT [128(dvA|dvB), S] via 512-col streams (+bias as a
      per-partition scalar add), then PE transposes (identity matmul) into
      the attn layout vh [128(t), k, 132] whose constant 128.0 denominator
      columns implement the softmax/(2*dk) scaling exactly (2*DK == 128).
  A   software-pipelined attention: scores+exp for tile (p, sq+1) are
      emitted interleaved with attn@V of tile (p, sq), so the Scalar
      engine's exp stream (the critical path, ~137us) never waits on
      attn@V's in-order PE traversal. scoresT = khT.T@qhT packs both heads
      via tile_position; attn@V carries a 65th denominator row; deferred
      normalization = reciprocal_approx_fast (after a partition-64->0 DMA
      hop; the custom DVE op mis-executes on partition bases >= 64) + PE
      broadcast + DVE mul. Pair-0's vproj fills the prologue, pair-1's
      rides the exp-bound slack one t-block per sq iteration.
  O   joint output projection: po = cat0.T@wo0 + cat1.T@wo1 accumulated in
      PSUM (K=128 per pair), PSUM->SBUF copies split Scalar/DVE, bf16
      partials summed on host.
"""

from contextlib import ExitStack

import numpy as np
import ml_dtypes

import concourse.bass as bass
import concourse.tile as tile
from concourse import bacc
from concourse import mybir

F32 = mybir.dt.float32
F32R = mybir.dt.float32r
BF16 = mybir.dt.bfloat16
EXP = mybir.ActivationFunctionType.Exp

B, S, D, NH, DK, DV = 2, 2048, 1024, 16, 64, 64
NCORES = 8
HPC = 4          # heads per core
NPAIR = 2        # head pairs per core


def build_nc(s=S, d=D):
    """Build the per-core Bass program (identical on all 8 cores)."""
    nc = bacc.Bacc("TRN2", target_bir_lowering=False, debug=False)

    sq_t = 512                  # sq tile (matmul free dim)
    n_sq = s // sq_t
    n_sk = s // 128             # sk tiles of 128
    n_ch = d // 128             # contraction chunks of 128

    qT_d = nc.dram_tensor("qT", [d, s], BF16, kind="ExternalInput").ap()
    kT_d = nc.dram_tensor("kT", [d, s], BF16, kind="ExternalInput").ap()
    vT_d = nc.dram_tensor("vT", [d, s], BF16, kind="ExternalInput").ap()
    wq_d = nc.dram_tensor("wq", [NPAIR, d, 128], BF16, kind="ExternalInput").ap()
    wk_d = nc.dram_tensor("wk", [NPAIR, d, 128], BF16, kind="ExternalInput").ap()
    wv_d = nc.dram_tensor("wv", [NPAIR, d, 128], BF16, kind="ExternalInput").ap()
    bqk_d = nc.dram_tensor("bqk", [128, 2 * NPAIR], F32, kind="ExternalInput").ap()
    bv_d = nc.dram_tensor("bv_col", [128, NPAIR], F32, kind="ExternalInput").ap()
    wo_d = nc.dram_tensor("wo", [NPAIR, 128, d], BF16, kind="ExternalInput").ap()
    onesr_d = nc.dram_tensor("onesr", [128, 64], F32R, kind="ExternalInput").ap()
    ident_d = nc.dram_tensor("ident", [128, 128], BF16, kind="ExternalInput").ap()
    out_d = nc.dram_tensor("out", [s, d], BF16, kind="ExternalOutput").ap()

    with tile.TileContext(nc) as tc, ExitStack() as ctx:
        consts = ctx.enter_context(tc.tile_pool(name="consts", bufs=1))
        qk_sb = ctx.enter_context(tc.tile_pool(name="qk_sb", bufs=1))
        vt_stream = ctx.enter_context(tc.tile_pool(name="vt_stream", bufs=12))
        vhT_pool = ctx.enter_context(tc.tile_pool(name="vhT", bufs=3))
        qkt_pool = ctx.enter_context(tc.tile_pool(name="qkt", bufs=1))
        vh_pool = ctx.enter_context(tc.tile_pool(name="vh", bufs=1))
        exp_pool = ctx.enter_context(tc.tile_pool(name="expp", bufs=16))
        cat_pool = ctx.enter_context(tc.tile_pool(name="cat", bufs=1))
        recip_pool = ctx.enter_context(tc.tile_pool(name="recip", bufs=2))
        out_pool = ctx.enter_context(tc.tile_pool(name="outp", bufs=3))
        ps = ctx.enter_context(tc.tile_pool(name="ps", bufs=4, space="PSUM"))

        # --- constants (merged DMAs; q/k-critical ones first) ---
        wq_sb = consts.tile([128, NPAIR, n_ch, 128], BF16, tag="wq")
        wk_sb = consts.tile([128, NPAIR, n_ch, 128], BF16, tag="wk")
        wv_sb = consts.tile([128, NPAIR, n_ch, 128], BF16, tag="wv")
        for p in range(NPAIR):
            nc.sync.dma_start(
                wq_sb[:, p], wq_d[p].rearrange("(c p) m -> p c m", p=128))
            nc.sync.dma_start(
                wk_sb[:, p], wk_d[p].rearrange("(c p) m -> p c m", p=128))
        bqk_sb = consts.tile([128, 2 * NPAIR], F32, tag="bqk")
        nc.sync.dma_start(bqk_sb[:], bqk_d[:])

        # ---- Phase L: stream q/k chunks split across BOTH hardware DGE
        # rings (even chunks one ring, odd the other) so each stream
        # arrives at ~2x single-ring rate; the Act ring's triggers all
        # retire well before the exp stream starts.
        qt_sb = qk_sb.tile([128, n_ch, s], BF16, tag="qt")
        kt_sb = qk_sb.tile([128, n_ch, s], BF16, tag="kt")
        for hs in range(2):
            ssl = bass.ts(hs, s // 2)
            for c in range(n_ch):
                csl = slice(c * 128, (c + 1) * 128)
                q_eng = nc.scalar if c % 2 == 0 else nc.sync
                k_eng = nc.sync if c % 2 == 0 else nc.scalar
                q_eng.dma_start(qt_sb[:, c, ssl], qT_d[csl, ssl])
                k_eng.dma_start(kt_sb[:, c, ssl], kT_d[csl, ssl])

        for p in range(NPAIR):
            nc.sync.dma_start(
                wv_sb[:, p], wv_d[p].rearrange("(c p) m -> p c m", p=128))
        bv_sb = consts.tile([128, NPAIR], F32, tag="bv")
        nc.sync.dma_start(bv_sb[:], bv_d[:])
        wo_sb = consts.tile([128, NPAIR, d], BF16, tag="wo")
        for p in range(NPAIR):
            nc.sync.dma_start(wo_sb[:, p, :], wo_d[p])
        ones_fr = consts.tile([128, 64], F32R, tag="ones_fr")
        nc.sync.dma_start(ones_fr[:], onesr_d[:])
        ident = consts.tile([128, 128], BF16, tag="ident")
        nc.sync.dma_start(ident[:], ident_d[:])

        # ---- Phase P1: q/k head projections for both pairs
        qhTs, khTs = [], []
        for p in range(NPAIR):
            qhT = qkt_pool.tile([128, s], BF16, tag=f"qhT{p}")
            khT = qkt_pool.tile([128, s], BF16, tag=f"khT{p}")
            qhTs.append(qhT)
            khTs.append(khT)

        vt2 = {}
        vt2b = {}

        def emit_scores_exp(qhT, khT, sq, k):
            ssl = bass.ts(sq, sq_t)
            ksl = bass.ts(k, 128)
            sAB = ps.tile([128, 2 * sq_t], F32, tag="ps2", bufs=2)
            nc.tensor.matmul(sAB[:, 0:sq_t], khT[0:64, ksl], qhT[0:64, ssl],
                             start=True, stop=True, tile_position=(0, 0))
            nc.tensor.matmul(sAB[:, sq_t:2 * sq_t], khT[64:128, ksl],
                             qhT[64:128, ssl],
                             start=True, stop=True, tile_position=(64, 0))
            eAB = exp_pool.tile([128, 2 * sq_t], BF16, tag="eAB")
            nc.scalar.activation(eAB[:], sAB[:], EXP)
            return eAB

        n_tt = s // 512

        # vh layout per k-tile: [vhA+bvA (64) | 128.0 | pad | vhB+bvB | 128.0 | pad]
        vhs = []
        for p in range(NPAIR):
            vh = vh_pool.tile([128, n_sk, 132], BF16, tag=f"vh{p}", name=f"vh{p}")
            # constant softmax-denominator columns (128 == 2*DK scaling)
            nc.gpsimd.memset(vh[:, :, 64:65], 128.0)
            nc.gpsimd.memset(vh[:, :, 130:131], 128.0)
            vhs.append(vh)

        def emit_vt_dmas(tt):
            tsl = bass.ts(tt, 512)
            vt_cs = []
            for c in range(n_ch):
                vt_c = vt_stream.tile([128, 512], BF16, tag="vt", name=f"vt{c}")
                nc.sync.dma_start(vt_c[:], vT_d[c * 128:(c + 1) * 128, tsl])
                vt_cs.append(vt_c)
            return vt_cs

        def emit_vproj(p, tt, vt_cs):
            psv = ps.tile([128, 512], F32, tag="ps", name=f"psv{p}")
            for c in range(n_ch):
                nc.tensor.matmul(psv[:], wv_sb[:, p, c, :], vt_cs[c][:],
                                 start=(c == 0), stop=(c == n_ch - 1))
            vsb = vhT_pool.tile([128, 512], BF16, tag="vsb")
            with nc.allow_low_precision(reason="bf16 rounding as baseline"):
                nc.vector.tensor_scalar_add(vsb[:], psv[:], bv_sb[:, p:p + 1])
            for j in range(4):
                k = tt * 4 + j
                # PE transpose [dv2, t128] -> [t128, dv2], writing the
                # gap layout (cols 0:64 head A, 65:129 head B); the
                # constant denominator columns are memset once above.
                vtr = ps.tile([128, 132], BF16, tag="ps", name="vtr")
                tr_dst = vtr.rearrange("p (b c) -> p b c", b=2)[:, :, 0:64]
                nc.tensor.transpose(tr_dst, vsb[:, j * 128:(j + 1) * 128],
                                    ident[:])
                dst = vhs[p][:, k, :].rearrange(
                    "p (b c) -> p b c", b=2)[:, :, 0:64]
                src = vtr.rearrange("p (b c) -> p b c", b=2)[:, :, 0:64]
                nc.vector.tensor_copy(dst, src)

        def emit_proj_half(p, half):
            ssl = bass.ts(half, sq_t)
            pq = ps.tile([128, sq_t], F32, tag="ps", name=f"psq{p}_{half}")
            pk = ps.tile([128, sq_t], F32, tag="ps", name=f"psk{p}_{half}")
            for c in range(n_ch):
                nc.tensor.matmul(pq[:], wq_sb[:, p, c, :], qt_sb[:, c, ssl],
                                 start=(c == 0), stop=(c == n_ch - 1))
                nc.tensor.matmul(pk[:], wk_sb[:, p, c, :], kt_sb[:, c, ssl],
                                 start=(c == 0), stop=(c == n_ch - 1))
            with nc.allow_low_precision(reason="bf16 rounding as baseline"):
                nc.vector.tensor_scalar_add(qhTs[p][:, ssl], pq[:],
                                            bqk_sb[:, 2 * p:2 * p + 1])
                nc.vector.tensor_scalar_add(khTs[p][:, ssl], pk[:],
                                            bqk_sb[:, 2 * p + 1:2 * p + 2])

        eABs = []
        for half in range(n_sq):
            emit_proj_half(0, half)

        # ---- Phase A: software-pipelined attention.
        # scores+exp for tile (p, sq+1) are emitted interleaved with the
        # attn@V consumption of tile (p, sq), so the Scalar engine's exp
        # stream (the critical path) never waits behind attn@V's in-order
        # PE traversal. v-projections ride in the leftover PE slack:
        # pair 0's in the prologue, pair 1's one t-block per sq iteration.
        cats = []
        for p in range(NPAIR):
            cats.append(cat_pool.tile([128, s], BF16, tag=f"cat{p}", name=f"cat{p}"))
        tiles = [(p, sq) for p in range(NPAIR) for sq in range(n_sq)]
        pending_norm = None

        # prologue: scores+exp for (0, 0) interleaved with pair-0 vproj;
        # the pair-1 q/k projections ride AFTER the first score batch so
        # the exp stream starts ~20us earlier (it only needs pair-0).
        for k in range(n_sk):
            if k % 4 == 0:
                vt2[k // 4] = emit_vt_dmas(k // 4)
            eABs.append(emit_scores_exp(qhTs[0], khTs[0], 0, k))
            if k % 4 == 3:
                emit_vproj(0, k // 4, vt2[k // 4])
        for half in range(n_sq):
            emit_proj_half(1, half)

        def emit_outproj(ot, o_anchor_box=[None]):
            osl = bass.ts(ot, 128)
            o_sb = out_pool.tile([128, d], BF16, tag="o", name=f"o{ot}")
            for dh in range(2):
                dsl = bass.ts(dh, 512)
                po = ps.tile([128, 512], F32, tag="ps", name=f"po{ot}_{dh}")
                mo = nc.tensor.matmul(po[:], cats[0][:, osl], wo_sb[:, 0, dsl],
                                      start=True, stop=False)
                nc.tensor.matmul(po[:], cats[1][:, osl], wo_sb[:, 1, dsl],
                                 start=False, stop=True)
                with nc.allow_low_precision(reason="bf16 partials, host sum"):
                    if dh == 0:
                        nc.scalar.copy(o_sb[:, dsl], po[:])
                    else:
                        nc.vector.tensor_copy(o_sb[:, dsl], po[:])
            # 4 parallel transfers: one 256KB dma_start runs on a single
            # DMA engine (~11.6us) and would dominate the kernel tail
            for q4 in range(4):
                rsl = slice(ot * 128 + q4 * 32, ot * 128 + (q4 + 1) * 32)
                nc.sync.dma_start(out_d[rsl, :], o_sb[q4 * 32:(q4 + 1) * 32, :])
            return mo

        for ti, (p, sq) in enumerate(tiles):
            qhT, khT, vh, cat = qhTs[p], khTs[p], vhs[p], cats[p]
            nxt = tiles[ti + 1] if ti + 1 < len(tiles) else None
            ssl = bass.ts(sq, sq_t)
            if p == 0:
                vt2b[sq] = emit_vt_dmas(sq)
            nA = ps.tile([128, sq_t], F32, tag="ps")
            nB = ps.tile([128, sq_t], F32, tag="ps")
            anchor = None
            next_eABs = []
            for k in range(n_sk):
                if nxt is not None:
                    next_eABs.append(
                        emit_scores_exp(qhTs[nxt[0]], khTs[nxt[0]],
                                        nxt[1], k))
                elif k >= 8:
                    # last tile: no more scores to pipeline; out-projection
                    # tiles whose cat columns are final (ot 0..7 need only
                    # cat sq0/sq1) ride the attn@V slack instead
                    emit_outproj(k - 8)
                eAB = eABs[k]
                nc.tensor.matmul(nA[0:65, :], vh[:, k, 0:65], eAB[:, 0:sq_t],
                                 start=(k == 0), stop=(k == n_sk - 1))
                mm_b = nc.tensor.matmul(nB[0:65, :], vh[:, k, 66:131],
                                        eAB[:, sq_t:2 * sq_t],
                                        start=(k == 0),
                                        stop=(k == n_sk - 1))
                if k == min(8, n_sk - 1):
                    anchor = mm_b
            eABs = next_eABs
            if pending_norm is not None:
                pending_norm(anchor)
                pending_norm = None
            # free nA/nB quickly: copy numerators + denominators out of
            # PSUM before the reciprocal runs.
            numAB = recip_pool.tile([64, 2 * sq_t], F32, tag="numAB")
            nc.vector.tensor_copy(numAB[:, 0:sq_t], nA[0:64, :])
            nc.vector.tensor_copy(numAB[:, sq_t:2 * sq_t], nB[0:64, :])
            den64 = recip_pool.tile([65, 2 * sq_t], F32, tag="den64")
            nc.vector.tensor_copy(den64[64:65, 0:sq_t], nA[64:65, :])
            nc.vector.tensor_copy(den64[64:65, sq_t:2 * sq_t], nB[64:65, :])
            rec = recip_pool.tile([1, 4 * sq_t], F32, tag="rec")
            # SBUF->SBUF partition move 64 -> 0: reciprocal_approx_fast
            # mis-executes on partition bases >= 64
            nc.sync.dma_start(rec[0:1, 0:2 * sq_t], den64[64:65, :])
            nc.vector.reciprocal_approx_fast(
                rec[0:1, 2 * sq_t:4 * sq_t], rec[0:1, 0:2 * sq_t])
            recr = recip_pool.tile([1, 2 * sq_t], F32R, tag="recr")
            with nc.allow_low_precision(reason="f32r == f32 bits"):
                nc.vector.tensor_copy(recr[0:1, :],
                                      rec[0:1, 2 * sq_t:4 * sq_t])

            def _normalize(anc, ssl=ssl, recr=recr, numAB=numAB, cat=cat):
                # deferred one sq-tile so the reciprocal latency hides
                # under the next k-loop instead of stalling the PE queue
                bcA = ps.tile([128, sq_t], F32, tag="ps", name="bcA")
                bcB = ps.tile([128, sq_t], F32, tag="ps", name="bcB")
                mA = nc.tensor.matmul(
                    bcA[0:64, :], ones_fr[0:1, :],
                    recr[0:1, 0:sq_t],
                    start=True, stop=True)
                if anc is not None:
                    tile.add_dep_helper(mA.ins, anc.ins, sync=False,
                                        reason="defer bcast past k-loop")
                nc.tensor.matmul(bcB[0:64, :], ones_fr[0:1, :],
                                 recr[0:1, sq_t:2 * sq_t],
                                 start=True, stop=True)
                nc.vector.tensor_mul(cat[0:64, ssl], bcA[0:64, :],
                                     numAB[:, 0:sq_t])
                nc.vector.tensor_mul(cat[64:128, ssl], bcB[0:64, :],
                                     numAB[:, sq_t:2 * sq_t])
            pending_norm = _normalize
            if p == 0:
                emit_vproj(1, sq, vt2b[sq])

        # ---- Phase O: remaining output projection (ot 0..7 rode the last
        # attention tile); only ot 12..15 depend on the last normalization
        o_anchor = None
        n_ot = s // 128
        for ot in range(8, n_ot):
            if pending_norm is not None and ot == 12:
                pending_norm(o_anchor)
                pending_norm = None
            mo = emit_outproj(ot)
            if ot == 10:
                o_anchor = mo
        if pending_norm is not None:
            pending_norm(None)
            pending_norm = None

    nc.compile()
    return nc


def make_core_inputs(Q, K, V, Wq, bq, Wk, bk, Wv, bv, Wo):
    """Host-side prep: transposes, casts, per-core weight packing."""
    bf = ml_dtypes.bfloat16
    QT = np.ascontiguousarray(
        np.transpose(np.asarray(Q, np.float32), (0, 2, 1))).astype(bf)
    KT = np.ascontiguousarray(
        np.transpose(np.asarray(K, np.float32), (0, 2, 1))).astype(bf)
    VT = np.ascontiguousarray(
        np.transpose(np.asarray(V, np.float32), (0, 2, 1))).astype(bf)

    in_maps = []
    for c in range(NCORES):
        bi = c // 4
        h0 = HPC * (c % 4)
        wq = np.stack([np.concatenate([Wq[h0 + 2 * p], Wq[h0 + 2 * p + 1]], 1)
                       for p in range(NPAIR)]).astype(np.float32).astype(bf)
        wk = np.stack([np.concatenate([Wk[h0 + 2 * p], Wk[h0 + 2 * p + 1]], 1)
                       for p in range(NPAIR)]).astype(np.float32).astype(bf)
        wv = np.stack([np.concatenate([Wv[h0 + 2 * p], Wv[h0 + 2 * p + 1]], 1)
                       for p in range(NPAIR)]).astype(np.float32).astype(bf)
        bqk = np.stack(
            [np.concatenate([bq[h0 + 2 * p], bq[h0 + 2 * p + 1]])
             if col == 0 else
             np.concatenate([bk[h0 + 2 * p], bk[h0 + 2 * p + 1]])
             for p in range(NPAIR) for col in range(2)],
            axis=1).astype(np.float32)
        bvc = np.stack(
            [np.concatenate([bv[h0 + 2 * p], bv[h0 + 2 * p + 1]])
             for p in range(NPAIR)], axis=1).astype(np.float32)
        wo = np.stack(
            [np.concatenate([Wo[64 * (h0 + 2 * p):64 * (h0 + 2 * p) + 64],
                             Wo[64 * (h0 + 2 * p + 1):64 * (h0 + 2 * p + 1) + 64]],
                            0)
             for p in range(NPAIR)]).astype(np.float32).astype(bf)
        in_maps.append({
            "qT": QT[bi], "kT": KT[bi], "vT": VT[bi],
            "wq": wq, "wk": wk, "wv": wv,
            "bqk": bqk, "bv_col": bvc, "wo": wo,
            "onesr": np.ones((128, 64), np.float32),
            "ident": np.eye(128, dtype=np.float32).astype(bf),
        })
    return in_maps


_NC_CACHE = {}


def _get_nc():
    if "nc" not in _NC_CACHE:
        _NC_CACHE["nc"] = build_nc()
    return _NC_CACHE["nc"]


def _install_ntff_hook_shim():
    """The agent image's antenv lacks axon_hooks; recreate the tiny
    get/set registry and register the ctypes NTFF profiler so trace=True
    can report HW exec time."""
    import sys
    import types
    if "antenv.axon_hooks" in sys.modules:
        return
    hook = None
    try:
        from trn_agent_boot.trn_boot import _ntff_profile_via_ctypes
        hook = _ntff_profile_via_ctypes("/opt/axon/libaxon_pjrt.so")
    except Exception:
        hook = None
    mod = types.ModuleType("antenv.axon_hooks")
    mod._hook = hook
    mod.get_axon_ntff_profile_hook = lambda: mod._hook
    mod.set_axon_ntff_profile_hook = lambda h: setattr(mod, "_hook", h)
    sys.modules["antenv.axon_hooks"] = mod


def kernel(Q, K, V, Wq, bq, Wk, bk, Wv, bv, Wo, bo, _trace=False):
    from concourse.bass_utils import run_bass_kernel_spmd

    if _trace:
        _install_ntff_hook_shim()

    nc = _get_nc()
    in_maps = make_core_inputs(Q, K, V, Wq, bq, Wk, bk, Wv, bv, Wo)
    res = None
    for attempt in range(3):
        try:
            res = run_bass_kernel_spmd(nc, in_maps, list(range(NCORES)),
                                       trace=_trace)
            break
        except Exception:
            # transient NRT_EXEC_UNIT_UNRECOVERABLE wedges recover on retry
            if attempt == 2:
                raise
    out = np.zeros((B, S, D), np.float32)
    for c, r in enumerate(res.results):
        out[c // 4] += np.asarray(r["out"]).astype(np.float32)
    out += np.asarray(bo, np.float32)[None, None, :]
    if _trace:
        return out, res
    return out



# revision 43
# speedup vs baseline: 1.0870x; 1.0870x over previous
"""Multi-head attention (nn_MultiHeadAttention) on 8 Trainium2 NeuronCores.

Hybrid batch x head sharding: core c owns batch c//4 and heads
4*(c%4)..4*(c%4)+3 (two head-PAIRS). Each core computes its 4 heads' full
attention plus the partial output projection for its batch; the host sums
the 4 partials per batch and adds bo.

Per-core kernel phases:
  P1  q/k projections for both pairs -> qhT/khT [128(dkA|dkB), S] bf16;
      qt streams on the Activation DGE queue, kt on Sync (parallel issue).
  P2  v projection as vhT [128(dvA|dvB), S] via 512-col streams (+bias as a
      per-partition scalar add), then PE transposes (identity matmul) into
      the attn layout vh [128(t), k, 132] whose constant 128.0 denominator
      columns implement the softmax/(2*dk) scaling exactly (2*DK == 128).
  A   software-pipelined attention: scores+exp for tile (p, sq+1) are
      emitted interleaved with attn@V of tile (p, sq), so the Scalar
      engine's exp stream (the critical path, ~137us) never waits on
      attn@V's in-order PE traversal. scoresT = khT.T@qhT packs both heads
      via tile_position; attn@V carries a 65th denominator row; deferred
      normalization = reciprocal_approx_fast (after a partition-64->0 DMA
      hop; the custom DVE op mis-executes on partition bases >= 64) + PE
      broadcast + DVE mul. Pair-0's vproj fills the prologue, pair-1's
      rides the exp-bound slack one t-block per sq iteration.
  O   joint output projection: po = cat0.T@wo0 + cat1.T@wo1 accumulated in
      PSUM (K=128 per pair), PSUM->SBUF copies split Scalar/DVE, bf16
      partials summed on host.
"""

from contextlib import ExitStack

import numpy as np
import ml_dtypes

import concourse.bass as bass
import concourse.tile as tile
from concourse import bacc
from concourse import mybir

F32 = mybir.dt.float32
F32R = mybir.dt.float32r
BF16 = mybir.dt.bfloat16
EXP = mybir.ActivationFunctionType.Exp

B, S, D, NH, DK, DV = 2, 2048, 1024, 16, 64, 64
NCORES = 8
HPC = 4          # heads per core
NPAIR = 2        # head pairs per core


def build_nc(s=S, d=D):
    """Build the per-core Bass program (identical on all 8 cores)."""
    nc = bacc.Bacc("TRN2", target_bir_lowering=False, debug=False)

    sq_t = 512                  # sq tile (matmul free dim)
    n_sq = s // sq_t
    n_sk = s // 128             # sk tiles of 128
    n_ch = d // 128             # contraction chunks of 128

    qT_d = nc.dram_tensor("qT", [d, s], BF16, kind="ExternalInput").ap()
    kT_d = nc.dram_tensor("kT", [d, s], BF16, kind="ExternalInput").ap()
    vT_d = nc.dram_tensor("vT", [d, s], BF16, kind="ExternalInput").ap()
    wq_d = nc.dram_tensor("wq", [NPAIR, d, 128], BF16, kind="ExternalInput").ap()
    wk_d = nc.dram_tensor("wk", [NPAIR, d, 128], BF16, kind="ExternalInput").ap()
    wv_d = nc.dram_tensor("wv", [NPAIR, d, 128], BF16, kind="ExternalInput").ap()
    bqk_d = nc.dram_tensor("bqk", [128, 2 * NPAIR], F32, kind="ExternalInput").ap()
    bv_d = nc.dram_tensor("bv_col", [128, NPAIR], F32, kind="ExternalInput").ap()
    wo_d = nc.dram_tensor("wo", [NPAIR, 128, d], BF16, kind="ExternalInput").ap()
    onesr_d = nc.dram_tensor("onesr", [128, 64], F32R, kind="ExternalInput").ap()
    ident_d = nc.dram_tensor("ident", [128, 128], BF16, kind="ExternalInput").ap()
    out_d = nc.dram_tensor("out", [s, d], BF16, kind="ExternalOutput").ap()

    with tile.TileContext(nc) as tc, ExitStack() as ctx:
        consts = ctx.enter_context(tc.tile_pool(name="consts", bufs=1))
        qk_sb = ctx.enter_context(tc.tile_pool(name="qk_sb", bufs=1))
        vt_stream = ctx.enter_context(tc.tile_pool(name="vt_stream", bufs=12))
        vhT_pool = ctx.enter_context(tc.tile_pool(name="vhT", bufs=3))
        qkt_pool = ctx.enter_context(tc.tile_pool(name="qkt", bufs=1))
        vh_pool = ctx.enter_context(tc.tile_pool(name="vh", bufs=1))
        exp_pool = ctx.enter_context(tc.tile_pool(name="expp", bufs=16))
        cat_pool = ctx.enter_context(tc.tile_pool(name="cat", bufs=1))
        recip_pool = ctx.enter_context(tc.tile_pool(name="recip", bufs=2))
        out_pool = ctx.enter_context(tc.tile_pool(name="outp", bufs=3))
        ps = ctx.enter_context(tc.tile_pool(name="ps", bufs=4, space="PSUM"))

        # --- constants (merged DMAs; q/k-critical ones first) ---
        wq_sb = consts.tile([128, NPAIR, n_ch, 128], BF16, tag="wq")
        wk_sb = consts.tile([128, NPAIR, n_ch, 128], BF16, tag="wk")
        wv_sb = consts.tile([128, NPAIR, n_ch, 128], BF16, tag="wv")
        for p in range(NPAIR):
            nc.sync.dma_start(
                wq_sb[:, p], wq_d[p].rearrange("(c p) m -> p c m", p=128))
            nc.sync.dma_start(
                wk_sb[:, p], wk_d[p].rearrange("(c p) m -> p c m", p=128))
        bqk_sb = consts.tile([128, 2 * NPAIR], F32, tag="bqk")
        nc.sync.dma_start(bqk_sb[:], bqk_d[:])

        # ---- Phase L: stream q/k chunks split across BOTH hardware DGE
        # rings (even chunks one ring, odd the other) so each stream
        # arrives at ~2x single-ring rate; the Act ring's triggers all
        # retire well before the exp stream starts.
        qt_sb = qk_sb.tile([128, n_ch, s], BF16, tag="qt")
        kt_sb = qk_sb.tile([128, n_ch, s], BF16, tag="kt")
        for hs in range(2):
            ssl = bass.ts(hs, s // 2)
            for c in range(n_ch):
                csl = slice(c * 128, (c + 1) * 128)
                nc.scalar.dma_start(qt_sb[:, c, ssl], qT_d[csl, ssl])
                nc.sync.dma_start(kt_sb[:, c, ssl], kT_d[csl, ssl])

        for p in range(NPAIR):
            nc.sync.dma_start(
                wv_sb[:, p], wv_d[p].rearrange("(c p) m -> p c m", p=128))
        bv_sb = consts.tile([128, NPAIR], F32, tag="bv")
        nc.sync.dma_start(bv_sb[:], bv_d[:])
        wo_sb = consts.tile([128, NPAIR, d], BF16, tag="wo")
        for p in range(NPAIR):
            nc.sync.dma_start(wo_sb[:, p, :], wo_d[p])
        ones_fr = consts.tile([128, 64], F32R, tag="ones_fr")
        nc.sync.dma_start(ones_fr[:], onesr_d[:])
        ident = consts.tile([128, 128], BF16, tag="ident")
        nc.sync.dma_start(ident[:], ident_d[:])

        # ---- Phase P1: q/k head projections for both pairs
        qhTs, khTs = [], []
        for p in range(NPAIR):
            qhT = qkt_pool.tile([128, s], BF16, tag=f"qhT{p}")
            khT = qkt_pool.tile([128, s], BF16, tag=f"khT{p}")
            qhTs.append(qhT)
            khTs.append(khT)

        vt2 = {}
        vt2b = {}

        def emit_scores_exp(qhT, khT, sq, k):
            ssl = bass.ts(sq, sq_t)
            ksl = bass.ts(k, 128)
            sAB = ps.tile([128, 2 * sq_t], F32, tag="ps2", bufs=2)
            nc.tensor.matmul(sAB[:, 0:sq_t], khT[0:64, ksl], qhT[0:64, ssl],
                             start=True, stop=True, tile_position=(0, 0))
            nc.tensor.matmul(sAB[:, sq_t:2 * sq_t], khT[64:128, ksl],
                             qhT[64:128, ssl],
                             start=True, stop=True, tile_position=(64, 0))
            eAB = exp_pool.tile([128, 2 * sq_t], BF16, tag="eAB")
            nc.scalar.activation(eAB[:], sAB[:], EXP)
            return eAB

        n_tt = s // 512

        # vh layout per k-tile: [vhA+bvA (64) | 128.0 | pad | vhB+bvB | 128.0 | pad]
        vhs = []
        for p in range(NPAIR):
            vh = vh_pool.tile([128, n_sk, 132], BF16, tag=f"vh{p}", name=f"vh{p}")
            # constant softmax-denominator columns (128 == 2*DK scaling)
            nc.gpsimd.memset(vh[:, :, 64:65], 128.0)
            nc.gpsimd.memset(vh[:, :, 130:131], 128.0)
            vhs.append(vh)

        def emit_vt_dmas(tt):
            tsl = bass.ts(tt, 512)
            vt_cs = []
            for c in range(n_ch):
                vt_c = vt_stream.tile([128, 512], BF16, tag="vt", name=f"vt{c}")
                nc.sync.dma_start(vt_c[:], vT_d[c * 128:(c + 1) * 128, tsl])
                vt_cs.append(vt_c)
            return vt_cs

        def emit_vproj(p, tt, vt_cs):
            psv = ps.tile([128, 512], F32, tag="ps", name=f"psv{p}")
            for c in range(n_ch):
                nc.tensor.matmul(psv[:], wv_sb[:, p, c, :], vt_cs[c][:],
                                 start=(c == 0), stop=(c == n_ch - 1))
            vsb = vhT_pool.tile([128, 512], BF16, tag="vsb")
            with nc.allow_low_precision(reason="bf16 rounding as baseline"):
                nc.vector.tensor_scalar_add(vsb[:], psv[:], bv_sb[:, p:p + 1])
            for j in range(4):
                k = tt * 4 + j
                # PE transpose [dv2, t128] -> [t128, dv2], writing the
                # gap layout (cols 0:64 head A, 65:129 head B); the
                # constant denominator columns are memset once above.
                vtr = ps.tile([128, 132], BF16, tag="ps", name="vtr")
                tr_dst = vtr.rearrange("p (b c) -> p b c", b=2)[:, :, 0:64]
                nc.tensor.transpose(tr_dst, vsb[:, j * 128:(j + 1) * 128],
                                    ident[:])
                dst = vhs[p][:, k, :].rearrange(
                    "p (b c) -> p b c", b=2)[:, :, 0:64]
                src = vtr.rearrange("p (b c) -> p b c", b=2)[:, :, 0:64]
                nc.vector.tensor_copy(dst, src)

        def emit_proj_half(p, half):
            ssl = bass.ts(half, sq_t)
            pq = ps.tile([128, sq_t], F32, tag="ps", name=f"psq{p}_{half}")
            pk = ps.tile([128, sq_t], F32, tag="ps", name=f"psk{p}_{half}")
            for c in range(n_ch):
                nc.tensor.matmul(pq[:], wq_sb[:, p, c, :], qt_sb[:, c, ssl],
                                 start=(c == 0), stop=(c == n_ch - 1))
                nc.tensor.matmul(pk[:], wk_sb[:, p, c, :], kt_sb[:, c, ssl],
                                 start=(c == 0), stop=(c == n_ch - 1))
            with nc.allow_low_precision(reason="bf16 rounding as baseline"):
                nc.vector.tensor_scalar_add(qhTs[p][:, ssl], pq[:],
                                            bqk_sb[:, 2 * p:2 * p + 1])
                nc.vector.tensor_scalar_add(khTs[p][:, ssl], pk[:],
                                            bqk_sb[:, 2 * p + 1:2 * p + 2])

        eABs = []
        for half in range(n_sq):
            emit_proj_half(0, half)

        # ---- Phase A: software-pipelined attention.
        # scores+exp for tile (p, sq+1) are emitted interleaved with the
        # attn@V consumption of tile (p, sq), so the Scalar engine's exp
        # stream (the critical path) never waits behind attn@V's in-order
        # PE traversal. v-projections ride in the leftover PE slack:
        # pair 0's in the prologue, pair 1's one t-block per sq iteration.
        cats = []
        for p in range(NPAIR):
            cats.append(cat_pool.tile([128, s], BF16, tag=f"cat{p}", name=f"cat{p}"))
        tiles = [(p, sq) for p in range(NPAIR) for sq in range(n_sq)]
        pending_norm = None

        # prologue: scores+exp for (0, 0) interleaved with pair-0 vproj;
        # the pair-1 q/k projections ride AFTER the first score batch so
        # the exp stream starts ~20us earlier (it only needs pair-0).
        for k in range(n_sk):
            if k % 4 == 0:
                vt2[k // 4] = emit_vt_dmas(k // 4)
            eABs.append(emit_scores_exp(qhTs[0], khTs[0], 0, k))
            if k % 4 == 3:
                emit_vproj(0, k // 4, vt2[k // 4])
        for half in range(n_sq):
            emit_proj_half(1, half)

        def emit_outproj(ot, o_anchor_box=[None]):
            osl = bass.ts(ot, 128)
            o_sb = out_pool.tile([128, d], BF16, tag="o", name=f"o{ot}")
            for dh in range(2):
                dsl = bass.ts(dh, 512)
                po = ps.tile([128, 512], F32, tag="ps", name=f"po{ot}_{dh}")
                mo = nc.tensor.matmul(po[:], cats[0][:, osl], wo_sb[:, 0, dsl],
                                      start=True, stop=False)
                nc.tensor.matmul(po[:], cats[1][:, osl], wo_sb[:, 1, dsl],
                                 start=False, stop=True)
                with nc.allow_low_precision(reason="bf16 partials, host sum"):
                    if dh == 0:
                        nc.scalar.copy(o_sb[:, dsl], po[:])
                    else:
                        nc.vector.tensor_copy(o_sb[:, dsl], po[:])
            nc.sync.dma_start(out_d[ot * 128:(ot + 1) * 128, :], o_sb[:])
            return mo

        for ti, (p, sq) in enumerate(tiles):
            qhT, khT, vh, cat = qhTs[p], khTs[p], vhs[p], cats[p]
            nxt = tiles[ti + 1] if ti + 1 < len(tiles) else None
            ssl = bass.ts(sq, sq_t)
            if p == 0:
                vt2b[sq] = emit_vt_dmas(sq)
            nA = ps.tile([128, sq_t], F32, tag="ps")
            nB = ps.tile([128, sq_t], F32, tag="ps")
            anchor = None
            next_eABs = []
            for k in range(n_sk):
                if nxt is not None:
                    next_eABs.append(
                        emit_scores_exp(qhTs[nxt[0]], khTs[nxt[0]],
                                        nxt[1], k))
                elif k >= 8:
                    # last tile: no more scores to pipeline; out-projection
                    # tiles whose cat columns are final (ot 0..7 need only
                    # cat sq0/sq1) ride the attn@V slack instead
                    emit_outproj(k - 8)
                eAB = eABs[k]
                nc.tensor.matmul(nA[0:65, :], vh[:, k, 0:65], eAB[:, 0:sq_t],
                                 start=(k == 0), stop=(k == n_sk - 1))
                mm_b = nc.tensor.matmul(nB[0:65, :], vh[:, k, 66:131],
                                        eAB[:, sq_t:2 * sq_t],
                                        start=(k == 0),
                                        stop=(k == n_sk - 1))
                if k == min(8, n_sk - 1):
                    anchor = mm_b
            eABs = next_eABs
            if pending_norm is not None:
                pending_norm(anchor)
                pending_norm = None
            # free nA/nB quickly: copy numerators + denominators out of
            # PSUM before the reciprocal runs.
            numAB = recip_pool.tile([64, 2 * sq_t], F32, tag="numAB")
            nc.vector.tensor_copy(numAB[:, 0:sq_t], nA[0:64, :])
            nc.vector.tensor_copy(numAB[:, sq_t:2 * sq_t], nB[0:64, :])
            den64 = recip_pool.tile([65, 2 * sq_t], F32, tag="den64")
            nc.vector.tensor_copy(den64[64:65, 0:sq_t], nA[64:65, :])
            nc.vector.tensor_copy(den64[64:65, sq_t:2 * sq_t], nB[64:65, :])
            rec = recip_pool.tile([1, 4 * sq_t], F32, tag="rec")
            # SBUF->SBUF partition move 64 -> 0: reciprocal_approx_fast
            # mis-executes on partition bases >= 64
            nc.sync.dma_start(rec[0:1, 0:2 * sq_t], den64[64:65, :])
            nc.vector.reciprocal_approx_fast(
                rec[0:1, 2 * sq_t:4 * sq_t], rec[0:1, 0:2 * sq_t])
            recr = recip_pool.tile([1, 2 * sq_t], F32R, tag="recr")
            with nc.allow_low_precision(reason="f32r == f32 bits"):
                nc.vector.tensor_copy(recr[0:1, :],
                                      rec[0:1, 2 * sq_t:4 * sq_t])

            def _normalize(anc, ssl=ssl, recr=recr, numAB=numAB, cat=cat):
                # deferred one sq-tile so the reciprocal latency hides
                # under the next k-loop instead of stalling the PE queue
                bcA = ps.tile([128, sq_t], F32, tag="ps", name="bcA")
                bcB = ps.tile([128, sq_t], F32, tag="ps", name="bcB")
                mA = nc.tensor.matmul(
                    bcA[0:64, :], ones_fr[0:1, :],
                    recr[0:1, 0:sq_t],
                    start=True, stop=True)
                if anc is not None:
                    tile.add_dep_helper(mA.ins, anc.ins, sync=False,
                                        reason="defer bcast past k-loop")
                nc.tensor.matmul(bcB[0:64, :], ones_fr[0:1, :],
                                 recr[0:1, sq_t:2 * sq_t],
                                 start=True, stop=True)
                nc.vector.tensor_mul(cat[0:64, ssl], bcA[0:64, :],
                                     numAB[:, 0:sq_t])
                nc.vector.tensor_mul(cat[64:128, ssl], bcB[0:64, :],
                                     numAB[:, sq_t:2 * sq_t])
            pending_norm = _normalize
            if p == 0:
                emit_vproj(1, sq, vt2b[sq])

        # ---- Phase O: remaining output projection (ot 0..7 rode the last
        # attention tile); only ot 12..15 depend on the last normalization
        o_anchor = None
        n_ot = s // 128
        for ot in range(8, n_ot):
            if pending_norm is not None and ot == 12:
                pending_norm(o_anchor)
                pending_norm = None
            mo = emit_outproj(ot)
            if ot == 10:
                o_anchor = mo
        if pending_norm is not None:
            pending_norm(None)
            pending_norm = None

    nc.compile()
    return nc


def make_core_inputs(Q, K, V, Wq, bq, Wk, bk, Wv, bv, Wo):
    """Host-side prep: transposes, casts, per-core weight packing."""
    bf = ml_dtypes.bfloat16
    QT = np.ascontiguousarray(
        np.transpose(np.asarray(Q, np.float32), (0, 2, 1))).astype(bf)
    KT = np.ascontiguousarray(
        np.transpose(np.asarray(K, np.float32), (0, 2, 1))).astype(bf)
    VT = np.ascontiguousarray(
        np.transpose(np.asarray(V, np.float32), (0, 2, 1))).astype(bf)

    in_maps = []
    for c in range(NCORES):
        bi = c // 4
        h0 = HPC * (c % 4)
        wq = np.stack([np.concatenate([Wq[h0 + 2 * p], Wq[h0 + 2 * p + 1]], 1)
                       for p in range(NPAIR)]).astype(np.float32).astype(bf)
        wk = np.stack([np.concatenate([Wk[h0 + 2 * p], Wk[h0 + 2 * p + 1]], 1)
                       for p in range(NPAIR)]).astype(np.float32).astype(bf)
        wv = np.stack([np.concatenate([Wv[h0 + 2 * p], Wv[h0 + 2 * p + 1]], 1)
                       for p in range(NPAIR)]).astype(np.float32).astype(bf)
        bqk = np.stack(
            [np.concatenate([bq[h0 + 2 * p], bq[h0 + 2 * p + 1]])
             if col == 0 else
             np.concatenate([bk[h0 + 2 * p], bk[h0 + 2 * p + 1]])
             for p in range(NPAIR) for col in range(2)],
            axis=1).astype(np.float32)
        bvc = np.stack(
            [np.concatenate([bv[h0 + 2 * p], bv[h0 + 2 * p + 1]])
             for p in range(NPAIR)], axis=1).astype(np.float32)
        wo = np.stack(
            [np.concatenate([Wo[64 * (h0 + 2 * p):64 * (h0 + 2 * p) + 64],
                             Wo[64 * (h0 + 2 * p + 1):64 * (h0 + 2 * p + 1) + 64]],
                            0)
             for p in range(NPAIR)]).astype(np.float32).astype(bf)
        in_maps.append({
            "qT": QT[bi], "kT": KT[bi], "vT": VT[bi],
            "wq": wq, "wk": wk, "wv": wv,
            "bqk": bqk, "bv_col": bvc, "wo": wo,
            "onesr": np.ones((128, 64), np.float32),
            "ident": np.eye(128, dtype=np.float32).astype(bf),
        })
    return in_maps


_NC_CACHE = {}


def _get_nc():
    if "nc" not in _NC_CACHE:
        _NC_CACHE["nc"] = build_nc()
    return _NC_CACHE["nc"]


def _install_ntff_hook_shim():
    """The agent image's antenv lacks axon_hooks; recreate the tiny
    get/set registry and register the ctypes NTFF profiler so trace=True
    can report HW exec time."""
    import sys
    import types
    if "antenv.axon_hooks" in sys.modules:
        return
    hook = None
    try:
        from trn_agent_boot.trn_boot import _ntff_profile_via_ctypes
        hook = _ntff_profile_via_ctypes("/opt/axon/libaxon_pjrt.so")
    except Exception:
        hook = None
    mod = types.ModuleType("antenv.axon_hooks")
    mod._hook = hook
    mod.get_axon_ntff_profile_hook = lambda: mod._hook
    mod.set_axon_ntff_profile_hook = lambda h: setattr(mod, "_hook", h)
    sys.modules["antenv.axon_hooks"] = mod


def kernel(Q, K, V, Wq, bq, Wk, bk, Wv, bv, Wo, bo, _trace=False):
    from concourse.bass_utils import run_bass_kernel_spmd

    if _trace:
        _install_ntff_hook_shim()

    nc = _get_nc()
    in_maps = make_core_inputs(Q, K, V, Wq, bq, Wk, bk, Wv, bv, Wo)
    res = None
    for attempt in range(3):
        try:
            res = run_bass_kernel_spmd(nc, in_maps, list(range(NCORES)),
                                       trace=_trace)
            break
        except Exception:
            # transient NRT_EXEC_UNIT_UNRECOVERABLE wedges recover on retry
            if attempt == 2:
                raise
    out = np.zeros((B, S, D), np.float32)
    for c, r in enumerate(res.results):
        out[c // 4] += np.asarray(r["out"]).astype(np.float32)
    out += np.asarray(bo, np.float32)[None, None, :]
    if _trace:
        return out, res
    return out

